# revision 45
# baseline (speedup 1.0000x reference)
"""BinaryNet MLP forward on 8 TRN2 NeuronCores.

Strategy: data-parallel over batch (2048 rows/core), feature-major on-chip
layout (activations stored [channel, batch]).  For layers 1-3 the positive
per-row weight scales and the BatchNorm variance cancel inside sign(), so
each layer reduces to:  g_l = 1{ A_l >= mean_batch(A_l) }  where
A_l = sign(W_l) @ h_{l-1} is an exact small integer computed with fp8 {+-1}
activations x fp8 {+-1} weights on the TensorEngine.  h_l is produced by
the Activation engine as Sign(A - mean) (integer margins >= 1/16384 make
the fp32 subtract sign-safe).  Layers 2-4 run fp8 DoubleRow (two
K-subtiles per pass).  Batch means come from four tiny (8KB) AllReduces
of per-channel column sums, computed by matvec tricks
(colsum(A_l) = sigma_l @ rowsum(h_{l-1})) so every layer needs only one
pass over its matmuls; the first few output tiles of each layer bridge
their psums to SBUF so the TensorE never stalls on the AllReduce.
Layer 1 (continuous x) uses a 2-term fp16 split of x (host-prepared),
which reproduces the reference's fp32 sign decisions exactly; layer 4
applies the real BatchNorm with weight scales.
"""
import sys, os
sys.path.insert(0, '/opt/trn_rl_repo')
import numpy as np
import ml_dtypes

import concourse.bass as bass
import concourse.bacc as bacc
import concourse.tile as tile
import concourse.mybir as mybir
from concourse import bass_utils

F32 = mybir.dt.float32
BF16 = mybir.dt.bfloat16
FP16 = mybir.dt.float16
FP8 = mybir.dt.float8e4
AF = mybir.ActivationFunctionType
ALU = mybir.AluOpType
AX = mybir.AxisListType
DR = mybir.MatmulPerfMode.DoubleRow

N_CORES = 8
D_IN, H, C = 784, 2048, 10
D_PAD = 896            # 7 * 128
KT1 = D_PAD // 128     # 7 k-tiles for layer 1
KT = H // 128          # 16 k-tiles for layers 2-4
NP = KT // 2           # 8 DoubleRow k-pairs
JT = H // 128          # 16 output-channel tiles
CHUNK = 512
BRIDGE_BUFS = 12       # SBUF bridge slots for psum->sbuf theta-decoupling
BRIDGE_T = 3           # j-tiles per layer whose psums get bridged
DRAIN_SPLIT = False    # odd j-tiles drain on DVE as {2,0}=h+1 (ACT/DVE balance)
MV_BATCH = False       # batched theta matvec: RHL stationary, sigma moving


def build(n_loc: int, single: bool = False, n_rep: int = 1):
    """Emit the SPMD program for one core (all 8 run it on their own shard).

    single=True builds a 1-core variant with AllReduces replaced by plain
    copies (for cost-model timeline analysis).  n_rep repeats the whole
    forward pass back-to-back (device-time benchmarking)."""
    nch = n_loc // CHUNK
    assert n_loc % CHUNK == 0
    inv_B = 1.0 / float(n_loc * N_CORES)   # exact: power of two
    inv_H = 1.0 / float(H)

    nc = bacc.Bacc("TRN2", target_bir_lowering=False, debug=False,
                   num_devices=1 if single else N_CORES)
    nc._single_fake_ar = single

    xh = nc.dram_tensor("xh", [D_PAD, n_loc], FP16, kind="ExternalInput")
    xl = nc.dram_tensor("xl", [D_PAD, n_loc], FP16, kind="ExternalInput")
    wT1 = nc.dram_tensor("wT1", [D_PAD, H], BF16, kind="ExternalInput")
    wT2 = nc.dram_tensor("wT2", [H, H], BF16, kind="ExternalInput")
    wT3 = nc.dram_tensor("wT3", [H, H], BF16, kind="ExternalInput")
    wT4 = nc.dram_tensor("wT4", [H, C], BF16, kind="ExternalInput")
    w4n = nc.dram_tensor("w4n", [C, H], F32, kind="ExternalInput")
    g4v = nc.dram_tensor("g4v", [C, 1], F32, kind="ExternalInput")
    b4v = nc.dram_tensor("b4v", [C, 1], F32, kind="ExternalInput")
    id2 = nc.dram_tensor("id2", [2, 2], F32, kind="ExternalInput") \
        if MV_BATCH else None
    yout = nc.dram_tensor("yout", [C, n_loc], F32, kind="ExternalOutput")

    xh_t = xh[:].rearrange("(t p) i -> t p i", p=128)
    xl_t = xl[:].rearrange("(t p) i -> t p i", p=128)
    wT1_t = wT1[:].rearrange("(t p) j -> t p j", p=128)
    wT2_t = wT2[:].rearrange("(t p) j -> t p j", p=128)
    wT3_t = wT3[:].rearrange("(t p) j -> t p j", p=128)
    wT4_t = wT4[:].rearrange("(t p) j -> t p j", p=128)

    with tile.TileContext(nc) as tc:
        for _rep in range(n_rep):
            _emit(tc, nc, n_loc, nch, inv_B, inv_H,
                  xh_t, xl_t, wT1_t, wT2_t, wT3_t, wT4_t, w4n, g4v, b4v, yout,
                  id2)
    nc.compile()
    return nc


def _emit(tc, nc, n_loc, nch, inv_B, inv_H,
          xh_t, xl_t, wT1_t, wT2_t, wT3_t, wT4_t, w4n, g4v, b4v, yout,
          id2=None):
    import contextlib
    es = contextlib.ExitStack()
    with es:
        misc = es.enter_context(tc.tile_pool(name="misc", bufs=1))
        dram = es.enter_context(tc.tile_pool(name="dram", bufs=1, space="DRAM"))
        wstage = es.enter_context(tc.tile_pool(name="wstage", bufs=2))
        ps_main = es.enter_context(tc.tile_pool(name="ps_main", bufs=6, space="PSUM"))
        ps_small = es.enter_context(tc.tile_pool(name="ps_small", bufs=2, space="PSUM"))
        # paired activation tiles [128, 2, n_loc]; g1/g3 rotate through p_gA,
        # g2 lives in p_gB (opened after the layer-1 pool closes)
        p_gA = es.enter_context(tc.tile_pool(name="p_gA", bufs=8))
        # sigma pool A: s1 (7x [128,H] fp8) + s2 (8 pairs, slots reused by s4)
        p_sA = es.enter_context(tc.tile_pool(name="p_sA", bufs=1))

        def allreduce(sbuf_src, shape, name):
            bi = dram.tile(shape, F32, name=f"{name}_bi", tag=f"{name}_bi")
            nc.sync.dma_start(bi[:], sbuf_src)
            dst = misc.tile(shape, F32, name=f"{name}_ar", tag=f"{name}_ar")
            if getattr(nc, "_single_fake_ar", False):
                nc.sync.dma_start(dst[:], bi[:])
                return dst
            bo = dram.tile(shape, F32, addr_space="Shared",
                           name=f"{name}_bo", tag=f"{name}_bo")
            nc.gpsimd.collective_compute(
                "AllReduce", ALU.add,
                replica_groups=[list(range(N_CORES))],
                ins=[bi.opt()], outs=[bo.opt()],
            )
            nc.sync.dma_start(dst[:], bo[:])
            return dst

        def hi_lo_interleave(vec_f32, ncols, name):
            # vec_f32: [128, ncols] f32 -> [128, ncols, 2] bf16 (hi, lo)
            hi = misc.tile([128, ncols], BF16, name=f"{name}_hi", tag=f"{name}_hi")
            nc.vector.tensor_copy(hi[:], vec_f32)
            lo = misc.tile([128, ncols], BF16, name=f"{name}_lo", tag=f"{name}_lo")
            nc.vector.scalar_tensor_tensor(lo[:], hi[:], -1.0, vec_f32,
                                           ALU.mult, ALU.add)
            hl = misc.tile([128, ncols, 2], BF16, name=f"{name}_hl", tag=f"{name}_hl")
            nc.vector.tensor_copy(hl[:, :, 0], hi[:])
            nc.vector.tensor_copy(hl[:, :, 1], lo[:])
            return hl

        def matvec_theta(sig_of, rhl, n_kt, name, sig_wide_of=None):
            # negated threshold: -mean_i A[j, i] (used as ACT Sign bias)
            theta = misc.tile([128, JT], F32, name=f"{name}", tag=f"{name}")
            if MV_BATCH and sig_wide_of is not None:
                # S[v, j] = sum_k rhl[k, v] * sigT[k, j]: RHL is the (2-col,
                # cheap-to-load) stationary; sigma streams at N=512.  Then
                # PE-transpose the [2, H] row pair into per-partition thetas.
                sbrow = misc.tile([2, H], F32, name=f"{name}_row",
                                  tag="mv_row", bufs=1)
                ident2 = misc.tile([2, 2], F32, name=f"{name}_id",
                                   tag="mv_ident", bufs=1)
                nc.sync.dma_start(ident2[:], id2[:])
                for q in range(H // CHUNK):
                    ps = ps_small.tile([2, CHUNK], F32,
                                       name=f"mv_{name}_q{q}", tag="ps_small")
                    for kt in range(n_kt):
                        nc.tensor.matmul(ps[:], rhl[:, kt, :],
                                         sig_wide_of(kt, q),
                                         start=(kt == 0),
                                         stop=(kt == n_kt - 1))
                    nc.vector.tensor_copy(sbrow[:, CHUNK*q:CHUNK*(q+1)], ps[:])
                for t in range(JT):
                    pst = ps_small.tile([128, 2], F32,
                                        name=f"mvT_{name}_{t}", tag="ps_small")
                    nc.tensor.transpose(pst[:], sbrow[:, 128*t:128*(t+1)],
                                        ident2[:])
                    nc.vector.tensor_reduce(theta[:, t:t+1], pst[:, 0:2],
                                            axis=AX.X, op=ALU.add)
            else:
                for t in range(JT):
                    ps = ps_small.tile([128, 2], F32, name=f"mv_{name}_{t}",
                                       tag="ps_small")
                    for kt in range(n_kt):
                        nc.tensor.matmul(ps[:], sig_of(kt, t), rhl[:, kt, :],
                                         start=(kt == 0), stop=(kt == n_kt - 1))
                    nc.vector.tensor_reduce(theta[:, t:t+1], ps[:, 0:2],
                                            axis=AX.X, op=ALU.add)
            thetas = misc.tile([128, JT], F32, name=f"{name}_s", tag=f"{name}_s")
            nc.vector.tensor_scalar_mul(thetas[:], theta[:], -inv_B)
            thp = misc.tile([128, JT], F32, name=f"{name}_p", tag=f"{name}_p")
            nc.vector.tensor_scalar_mul(thp[:], theta[:], inv_B)
            return thetas, thp

        def sign_prep_paired(w_t, width, pool, tag, name):
            pairs = []
            for m in range(NP):
                sg = pool.tile([128, 2, width], FP8, name=f"{name}_{m}",
                               tag=tag, bufs=NP)
                for p in (0, 1):
                    kt = 2*m + p
                    st = wstage.tile([128, width], BF16,
                                     name=f"wst_{name}_{kt}", tag="wstage")
                    nc.sync.dma_start(st[:], w_t[kt])
                    nc.scalar.activation(sg[:, p, :], st[:], AF.Sign)
                pairs.append(sg)
            return pairs

        def drains(gp_of, t, pss, thn, r, accs_tag, lname, bridge_pool=None,
                   thp=None):
            # h = Sign(A - mean) in {-1,+1} fp8, on the Activation engine
            # (margins are >= 1/16384 with |A| << 1024, so the fp32 subtract
            # never rounds to exactly 0 and Sign never emits 0).
            # With DRAIN_SPLIT, odd j-tiles instead store h+1 in {2,0} via a
            # DVE is_ge: per-tile affine encodings cancel in the mean-compare.
            on_dve = DRAIN_SPLIT and (t % 2 == 1) and thp is not None
            accs = misc.tile([128, nch], F32, name=f"acc_{lname}_{t}",
                             tag=accs_tag, bufs=4) if r is not None else None
            srcs = pss
            if bridge_pool is not None:
                # copy psums to SBUF right away (no theta dep) so the banks
                # free up while the AllReduce for theta is still in flight
                srcs = []
                for c in range(nch):
                    tb = bridge_pool.tile([128, CHUNK], F32,
                                          name=f"br_{lname}_{t}_{c}",
                                          tag="bridge", bufs=BRIDGE_BUFS)
                    nc.vector.tensor_copy(tb[:], pss[c])
                    srcs.append(tb)
            for c in range(nch):
                sl = gp_of(t, c)
                if on_dve:
                    if r is not None:
                        nc.vector.tensor_scalar(sl, srcs[c], thp[:, t:t+1],
                                                None, ALU.is_ge, ALU.add,
                                                accum_out=accs[:, c:c+1])
                        nc.vector.tensor_scalar(sl, sl, 2.0, None, ALU.mult)
                    else:
                        nc.vector.tensor_scalar(sl, srcs[c], thp[:, t:t+1],
                                                2.0, ALU.is_ge, ALU.mult)
                elif r is not None:
                    nc.scalar.activation(sl, srcs[c], AF.Sign,
                                         bias=thn[:, t:t+1],
                                         accum_out=accs[:, c:c+1])
                else:
                    nc.scalar.activation(sl, srcs[c], AF.Sign,
                                         bias=thn[:, t:t+1])
            if r is not None:
                nc.vector.tensor_reduce(r[:, t:t+1], accs[:], axis=AX.X,
                                        op=ALU.add)
                if on_dve:  # stored values are 2*count; r must sum them
                    nc.vector.tensor_scalar_mul(r[:, t:t+1], r[:, t:t+1], 2.0)

        def alloc_g_pairs(pool, tag, lname):
            return [pool.tile([128, 2, n_loc], FP8, name=f"g_{lname}_{m}",
                              tag=tag) for m in range(NP)]

        def layer_dr(sig_pairs, gin_pairs, theta, gout_pairs, want_r, lname,
                     bridge_pool=None, bridge_t=None, thp=None):
            if bridge_t is None:
                bridge_t = BRIDGE_T
            # DoubleRow fp8 layer: A = sigma @ (prev g), drained to is_ge
            r = misc.tile([128, JT], F32, name=f"r_{lname}", tag=f"r_{lname}") \
                if want_r else None
            for t in range(JT):
                pss = [ps_main.tile([128, CHUNK], F32,
                                    name=f"ps_{lname}_{t}_{c}", tag="ps_main")
                       for c in range(nch)]
                for m in range(NP):
                    lhs = sig_pairs[m][:, :, 128*t:128*(t+1)]
                    for c in range(nch):
                        nc.tensor.matmul(pss[c], lhs,
                                         gin_pairs[m][:, :, CHUNK*c:CHUNK*(c+1)],
                                         start=(m == 0), stop=(m == NP - 1),
                                         perf_mode=DR)
                drains(lambda tt, cc: gout_pairs[tt//2][:, tt % 2,
                                                        CHUNK*cc:CHUNK*(cc+1)],
                       t, pss, theta, r, "accs", lname,
                       bridge_pool=bridge_pool if t < bridge_t else None,
                       thp=thp)
            return r

        # ---------------- layer 1: x load (chunked), sums, hi/lo/lo2 --------
        pl1_cm = tc.tile_pool(name="pl1", bufs=1)
        pl1 = pl1_cm.__enter__()

        # x arrives pre-split into fp16 hi/lo (host-side dtype marshaling);
        # local column sums xs = reduce(hi) + reduce(lo)
        xs_loc = misc.tile([128, KT1], F32, name="xs_loc", tag="xs_loc")
        xs_tmp = misc.tile([128, 1], F32, name="xs_tmp", tag="xs_tmp", bufs=2)
        n_terms = 2
        xterms = [[], []]
        for kt in range(KT1):
            hi = pl1.tile([128, n_loc], FP16, name=f"xt0_{kt}", tag="xhl",
                          bufs=14)
            lo = pl1.tile([128, n_loc], FP16, name=f"xt1_{kt}", tag="xhl",
                          bufs=14)
            nc.sync.dma_start(hi[:], xh_t[kt])
            nc.sync.dma_start(lo[:], xl_t[kt])
            nc.vector.tensor_reduce(xs_loc[:, kt:kt+1], hi[:], axis=AX.X,
                                    op=ALU.add)
            nc.vector.tensor_reduce(xs_tmp[:], lo[:], axis=AX.X, op=ALU.add)
            nc.vector.tensor_add(xs_loc[:, kt:kt+1], xs_loc[:, kt:kt+1],
                                 xs_tmp[:])
            xterms[0].append(hi)
            xterms[1].append(lo)

        # sign(w1): 7 unpaired fp8 tiles (layer 1 is bf16 x fp8, no DoubleRow)
        s1_tiles = []
        for kt in range(KT1):
            st = wstage.tile([128, H], BF16, name=f"wst_s1_{kt}", tag="wstage")
            nc.sync.dma_start(st[:], wT1_t[kt])
            sg = p_sA.tile([128, H], FP8, name=f"s1_{kt}", tag="s1", bufs=KT1)
            nc.scalar.activation(sg[:], st[:], AF.Sign)
            s1_tiles.append(sg)

        XS = allreduce(xs_loc[:], [128, KT1], "xs")
        xshl = hi_lo_interleave(XS[:], KT1, "xs")
        theta1, thp1 = matvec_theta(
            lambda kt, t: s1_tiles[kt][:, 128*t:128*(t+1)], xshl, KT1, "th1",
            sig_wide_of=lambda kt, q: s1_tiles[kt][:, CHUNK*q:CHUNK*(q+1)])

        # sigma2 prep (ACT/DMA run concurrently with layer-1 matmuls)
        s2_pairs = sign_prep_paired(wT2_t, H, p_sA, "s2", "s2")

        # ---------------- layer 1 main (3-term bf16 x fp8 sign) -------------
        g1_pairs = alloc_g_pairs(p_gA, "gA", "l1")
        r1 = misc.tile([128, JT], F32, name="r_l1", tag="r_l1")
        for t in range(JT):
            pss = [ps_main.tile([128, CHUNK], F32, name=f"ps_l1_{t}_{c}",
                                tag="ps_main") for c in range(nch)]
            for kt in range(KT1):
                lhs = s1_tiles[kt][:, 128*t:128*(t+1)]
                for c in range(nch):
                    for v in range(n_terms):
                        src = xterms[v][kt]
                        nc.tensor.matmul(pss[c], lhs,
                                         src[:, CHUNK*c:CHUNK*(c+1)],
                                         start=(kt == 0 and v == 0),
                                         stop=(kt == KT1 - 1 and
                                               v == n_terms - 1))
            drains(lambda tt, cc: g1_pairs[tt//2][:, tt % 2,
                                                  CHUNK*cc:CHUNK*(cc+1)],
                   t, pss, theta1, r1, "accs", "l1",
                   bridge_pool=pl1 if t < 3 else None, thp=thp1)

        pl1_cm.__exit__(None, None, None)

        p_gB = es.enter_context(tc.tile_pool(name="p_gB", bufs=8))
        p_sB = es.enter_context(tc.tile_pool(name="p_sB", bufs=1))
        brB_cm = tc.tile_pool(name="brB", bufs=1)
        brB = brB_cm.__enter__()

        R1 = allreduce(r1[:], [128, JT], "r1")
        rhl1 = hi_lo_interleave(R1[:], JT, "r1")
        theta2, thp2 = matvec_theta(
            lambda kt, t: s2_pairs[kt//2][:, kt % 2, 128*t:128*(t+1)],
            rhl1, KT, "th2",
            sig_wide_of=lambda kt, q: s2_pairs[kt//2][:, kt % 2,
                                               CHUNK*q:CHUNK*(q+1)])

        s3_pairs = sign_prep_paired(wT3_t, H, p_sB, "s3", "s3")

        g2_pairs = alloc_g_pairs(p_gB, "gB", "l2")
        r2 = layer_dr(s2_pairs, g1_pairs, theta2, g2_pairs, True, "l2",
                      bridge_pool=brB, thp=thp2)

        R2 = allreduce(r2[:], [128, JT], "r2")
        rhl2 = hi_lo_interleave(R2[:], JT, "r2")
        theta3, thp3 = matvec_theta(
            lambda kt, t: s3_pairs[kt//2][:, kt % 2, 128*t:128*(t+1)],
            rhl2, KT, "th3",
            sig_wide_of=lambda kt, q: s3_pairs[kt//2][:, kt % 2,
                                               CHUNK*q:CHUNK*(q+1)])

        # sigma4: DoubleRow LDWEIGHTS needs 16-aligned plane width -> pad C to 16
        s4_pairs = []
        for m in range(NP):
            sg = p_sA.tile([128, 2, 16], FP8, name=f"s4_{m}", tag="s2", bufs=NP)
            nc.vector.memset(sg[:], 0.0)
            for p in (0, 1):
                kt = 2*m + p
                st4 = wstage.tile([128, C], BF16, name=f"wst_s4_{kt}",
                                  tag="wstage")
                nc.sync.dma_start(st4[:], wT4_t[kt])
                nc.scalar.activation(sg[:, p, 0:C], st4[:], AF.Sign)
            s4_pairs.append(sg)

        g3_pairs = alloc_g_pairs(p_gA, "gA", "l3")
        layer_dr(s3_pairs, g2_pairs, theta3, g3_pairs, False, "l3",
                 bridge_pool=brB, thp=thp3)

        brB_cm.__exit__(None, None, None)
        p_l4 = es.enter_context(tc.tile_pool(name="p_l4", bufs=1))

        # ---------------- layer 4 + BatchNorm ----------------
        w4sb = p_l4.tile([C, H], F32, name="w4sb", tag="w4sb")
        nc.sync.dma_start(w4sb[:], w4n[:])
        g4sb = misc.tile([C, 1], F32, name="g4sb", tag="g4sb")
        nc.sync.dma_start(g4sb[:], g4v[:])
        b4sb = misc.tile([C, 1], F32, name="b4sb", tag="b4sb")
        nc.sync.dma_start(b4sb[:], b4v[:])

        s4raw = misc.tile([C, 1], F32, name="s4raw", tag="s4raw")
        nc.vector.tensor_reduce(s4raw[:], w4sb[:], axis=AX.X, op=ALU.add,
                                apply_absolute_value=True)
        s4 = misc.tile([C, 1], F32, name="s4", tag="s4")
        nc.vector.tensor_scalar_mul(s4[:], s4raw[:], inv_H)     # mean|w4|

        y4 = p_l4.tile([C, n_loc], F32, name="y4", tag="y4")
        for c in range(nch):
            ps = ps_small.tile([16, CHUNK], F32, name=f"ps_l4_{c}", tag="ps_small")
            for m in range(NP):
                nc.tensor.matmul(ps[:], s4_pairs[m][:],
                                 g3_pairs[m][:, :, CHUNK*c:CHUNK*(c+1)],
                                 start=(m == 0), stop=(m == NP - 1),
                                 perf_mode=DR)
            # y4 = s4 * k4  (h in {-1,+1} so A4 is k4 directly)
            nc.vector.tensor_scalar(y4[:, CHUNK*c:CHUNK*(c+1)], ps[0:C, :],
                                    s4[:], None, ALU.mult)

        ysum = misc.tile([C, 1], F32, name="ysum", tag="ysum")
        nc.vector.tensor_reduce(ysum[:], y4[:], axis=AX.X, op=ALU.add)
        ysq = p_l4.tile([C, n_loc], F32, name="ysq", tag="l4scratch")
        nc.vector.tensor_mul(ysq[:], y4[:], y4[:])
        ysqsum = misc.tile([C, 1], F32, name="ysqsum", tag="ysqsum")
        nc.vector.tensor_reduce(ysqsum[:], ysq[:], axis=AX.X, op=ALU.add)
        p4 = misc.tile([C, 2], F32, name="p4", tag="p4")
        nc.vector.tensor_copy(p4[:, 0:1], ysum[:])
        nc.vector.tensor_copy(p4[:, 1:2], ysqsum[:])

        G4 = allreduce(p4[:], [C, 2], "p4")
        mu4 = misc.tile([C, 1], F32, name="mu4", tag="mu4")
        nc.vector.tensor_scalar_mul(mu4[:], G4[:, 0:1], inv_B)
        ey2 = misc.tile([C, 1], F32, name="ey2", tag="ey2")
        nc.vector.tensor_scalar_mul(ey2[:], G4[:, 1:2], inv_B)
        mu4sq = misc.tile([C, 1], F32, name="mu4sq", tag="mu4sq")
        nc.vector.tensor_mul(mu4sq[:], mu4[:], mu4[:])
        var4 = misc.tile([C, 1], F32, name="var4", tag="var4")
        nc.vector.tensor_sub(var4[:], ey2[:], mu4sq[:])
        veps = misc.tile([C, 1], F32, name="veps", tag="veps")
        nc.vector.tensor_scalar_add(veps[:], var4[:], 1e-5)
        sd = misc.tile([C, 1], F32, name="sd", tag="sd")
        nc.scalar.activation(sd[:], veps[:], AF.Sqrt)
        inv_sd = misc.tile([C, 1], F32, name="inv_sd", tag="inv_sd")
        nc.vector.reciprocal(inv_sd[:], sd[:])
        alpha = misc.tile([C, 1], F32, name="alpha", tag="alpha")
        nc.vector.tensor_mul(alpha[:], inv_sd[:], g4sb[:])

        yo = p_l4.tile([C, n_loc], F32, name="yo", tag="yo")
        nc.vector.tensor_scalar(yo[:], y4[:], mu4[:], alpha[:],
                                ALU.subtract, ALU.mult)
        nc.vector.tensor_scalar(yo[:], yo[:], b4sb[:], None, ALU.add)
        nc.sync.dma_start(yout[:], yo[:])


# --------------------------------------------------------------------------
def prep_inputs(x, w1, w2, w3, w4, g4, b4, n_loc):
    bf16 = ml_dtypes.bfloat16
    n_cores = N_CORES
    B = x.shape[0]
    assert B == n_loc * n_cores

    f16 = np.float16
    wT1 = np.zeros((D_PAD, H), dtype=bf16)
    wT1[:D_IN] = w1.T.astype(bf16)
    wT2 = np.ascontiguousarray(w2.T.astype(bf16))
    wT3 = np.ascontiguousarray(w3.T.astype(bf16))
    wT4 = np.ascontiguousarray(w4.T.astype(bf16))
    w4n = np.ascontiguousarray(w4.astype(np.float32))
    g4v = np.ascontiguousarray(g4.reshape(C, 1).astype(np.float32))
    b4v = np.ascontiguousarray(b4.reshape(C, 1).astype(np.float32))

    in_maps = []
    for cidx in range(n_cores):
        xs = x[n_loc*cidx:n_loc*(cidx+1)]
        xT = np.zeros((D_PAD, n_loc), dtype=np.float32)
        xT[:D_IN] = xs.T
        xhp = xT.astype(f16)
        xlp = (xT - xhp.astype(np.float32)).astype(f16)
        m = {
            "xh": xhp, "xl": xlp, "wT1": wT1, "wT2": wT2, "wT3": wT3,
            "wT4": wT4, "w4n": w4n, "g4v": g4v, "b4v": b4v,
        }
        if MV_BATCH:
            m["id2"] = np.eye(2, dtype=np.float32)
        in_maps.append(m)
    return in_maps


_NC_CACHE = {}


def kernel(x, w1, w2, w3, w4, g1, b1, g2, b2, g3, b3, g4, b4):
    x = np.asarray(x); w1 = np.asarray(w1); w2 = np.asarray(w2)
    w3 = np.asarray(w3); w4 = np.asarray(w4)
    g4 = np.asarray(g4); b4 = np.asarray(b4)
    # layers 1-3 BN params: scales cancel inside sign() only when gamma>0, beta=0
    for g in (g1, g2, g3):
        assert np.all(np.asarray(g) > 0), "kernel assumes gamma > 0 for hidden BNs"
    for b in (b1, b2, b3):
        assert np.all(np.asarray(b) == 0), "kernel assumes beta == 0 for hidden BNs"
    for w in (w1, w2, w3, w4):
        assert not np.any(w == 0.0), "exact-zero weight would break Sign()"

    n_loc = x.shape[0] // N_CORES
    if n_loc not in _NC_CACHE:
        _NC_CACHE[n_loc] = build(n_loc)
    nc = _NC_CACHE[n_loc]

    in_maps = prep_inputs(x, w1, w2, w3, w4, g4, b4, n_loc)
    res = bass_utils.run_bass_kernel_spmd(nc, in_maps,
                                          core_ids=list(range(N_CORES)))
    out = np.concatenate([res.results[c]["yout"].T for c in range(N_CORES)],
                         axis=0)
    return out.astype(np.float32)
